# revision 49
# baseline (speedup 1.0000x reference)
"""Multihead attention (B=2, S=2048, D=1024, 16 heads) on 8 trn2 NeuronCores.

Sharding: data-parallel over batch (2 groups of 4 cores), tensor-parallel over
heads within a group (4 heads/core, W_q/W_k/W_v column-sliced, W_o row-sliced).
Each core returns a partial [2048, 1024] output; the host sums the 4 partials
per batch and adds the constant row bv @ Wo + bo (the V-bias contribution is
constant because softmax rows sum to 1).

Key optimizations over the 290-380us baseline:
  * KV compaction: the key-padding mask drops ~50% of keys.  The host packs
    the unmasked keys (order-preserving, exact w.r.t. the reference softmax)
    and pads to a multiple of 128; all kv-dimension work (K/V projections,
    scores, exp, PV) shrinks from 16 to NKT (9 for the seed-0 mask) tiles.
    Padding rows project to K=bk -> finite scores, and are excluded from both
    numerator and denominator by the multiplicative mask column in V.
  * bf16 operands everywhere (inputs, weights, Q/K/V, P, O): halves DMA and
    SBUF traffic; PSUM accumulation stays fp32.  bf16 matmuls run 1 cyc/row
    at any moving-dim size (fp32r needs N>=256).
  * Paired scores: even/odd heads of a pair occupy PE rows 0-63 / 64-127
    (K=64 row-group tiling -> the two matmuls run concurrently), writing one
    [128, 2, 512] PSUM tile so exp batches 1024 elements per ACT instruction.
  * Software pipelining: the PV + out-proj matmuls of the previous head-pair
    are emitted between the score matmuls of the current pair, so the PE
    queue always has ready work while ACT (the bottleneck: exp is
    ~4.3M elem/core even after compaction) drains the exp stream.
"""

import math
from contextlib import ExitStack

import numpy as np

import concourse.bacc as bacc
import concourse.tile as tile
import concourse.mybir as mybir
from concourse.bass_utils import run_bass_kernel_spmd

F32 = mybir.dt.float32
F32R = mybir.dt.float32r
BF16 = mybir.dt.bfloat16
EXP = mybir.ActivationFunctionType.Exp
MULT = mybir.AluOpType.mult

B, SQ, SKV = 2, 2048, 2048
D, NH, HD = 1024, 16, 64
NCORES = 8
HPC = NH // (NCORES // B)     # 4 heads per core
CS = HPC * HD                 # 256 projection columns per core
QC = 512                      # q chunk
NQC = SQ // QC                # 4 q chunks
NDT = D // 128                # 8 contraction tiles
NST = SQ // 128               # 16 output row tiles


def _build(nkt: int, loop_n: int = 1, interleave: bool = True,
           p_bufs: int = 2, copy_eng: str = "vector", phases: str = "12",
           vdefer: bool = True, bigdma: int = 0, ringsplit: bool = False,
           obf16: bool = False, actbias: bool = False, fuselast: bool = True):
    if nkt > 12:
        p_bufs = 1  # SBUF headroom at large kv (P is 2KB/partition per tile)
    skv = nkt * 128
    nc = bacc.Bacc(None, target_bir_lowering=False)
    xT = nc.dram_tensor("xT", [D, SQ], BF16, kind="ExternalInput")
    kvT = nc.dram_tensor("kvT", [D, skv], BF16, kind="ExternalInput")
    wq = nc.dram_tensor("wq", [128, NDT, CS], BF16, kind="ExternalInput")
    wk = nc.dram_tensor("wk", [128, NDT, CS], BF16, kind="ExternalInput")
    wv = nc.dram_tensor("wv", [128, NDT, CS], BF16, kind="ExternalInput")
    wo = nc.dram_tensor("wo", [128, 2, D], BF16, kind="ExternalInput")
    bqk = nc.dram_tensor("bqk", [128, 4], F32, kind="ExternalInput")
    mcol = nc.dram_tensor("mcol", [128, nkt], F32, kind="ExternalInput")
    mones = nc.dram_tensor("mones", [128, nkt, HPC], BF16, kind="ExternalInput")
    out_p = nc.dram_tensor("out_p", [SQ, D], BF16 if obf16 else F32,
                           kind="ExternalOutput")

    with tile.TileContext(nc) as tc:
        with tc.tile_pool(name="const", bufs=1) as const, \
             tc.tile_pool(name="big", bufs=1) as big:
            wq_sb = const.tile([128, NDT, CS], BF16)
            wk_sb = const.tile([128, NDT, CS], BF16)
            wv_sb = const.tile([128, NDT, CS], BF16)
            wo_sb = const.tile([128, 2, D], BF16)
            bqk_sb = const.tile([128, 4], F32)
            mcol_sb = const.tile([128, nkt], F32)
            nc.gpsimd.dma_start(out=wq_sb, in_=wq[:, :, :])
            nc.gpsimd.dma_start(out=bqk_sb, in_=bqk[:, :])
            nc.gpsimd.dma_start(out=wk_sb, in_=wk[:, :, :])
            nc.gpsimd.dma_start(out=wv_sb, in_=wv[:, :, :])
            nc.gpsimd.dma_start(out=mcol_sb, in_=mcol[:, :])
            nc.gpsimd.dma_start(out=wo_sb, in_=wo[:, :, :])

            QT = big.tile([128, 2, SQ], BF16)        # [hd(2x64), mh, q]
            KT = big.tile([128, 2, skv], BF16)       # [hd(2x64), mh, kv]
            V = big.tile([128, nkt, HPC, HD + 1], BF16)  # V in 0:64, mask col 64
            OT = big.tile([128, 2, SQ], BF16)        # [c(2x128), ct, q]

            if loop_n > 1:
                loop_cm = tc.For_i(0, loop_n, 1)
                loop_cm.__enter__()

            nc.gpsimd.dma_start(out=V[:, :, :, HD:HD + 1], in_=mones[:, :, :])

            # ---- Phase 1: projections ----
            # xT streams per-dq-tile (SP ring); kvT loads resident (ACT ring).
            dma_only = "d" in phases
            with ExitStack() as _ph1:
              if "1" in phases:
                xin = _ph1.enter_context(tc.tile_pool(name="xin", bufs=8 if not bigdma else 1))
                kvin = _ph1.enter_context(tc.tile_pool(name="kvin", bufs=1))
                if bigdma:
                    # >=1MB transfers (completion-latency amortization); rows
                    # regrouped so partition p, slot dt reads DRAM row
                    # dt*128+p.  bigdma = number of slabs per tensor.
                    g = NDT // bigdma
                    kvts, xts = [], []
                    for s in range(bigdma):
                        kv_sl = kvin.tile([128, g, skv], BF16, tag=f"kvs{s}",
                                          name=f"kv_sl{s}")
                        nc.scalar.dma_start(
                            out=kv_sl,
                            in_=kvT[s * g * 128:(s + 1) * g * 128, :]
                            .rearrange("(a p) n -> p a n", p=128))
                        kvts += [kv_sl[:, j, :] for j in range(g)]
                        x_sl = xin.tile([128, g, SQ], BF16, tag=f"xs{s}",
                                        name=f"x_sl{s}")
                        nc.sync.dma_start(
                            out=x_sl,
                            in_=xT[s * g * 128:(s + 1) * g * 128, :]
                            .rearrange("(a p) n -> p a n", p=128))
                        xts += [x_sl[:, j, :] for j in range(g)]
                else:
                    kvts = []
                    for dt in range(NDT):
                        kvt_t = kvin.tile([128, skv], BF16, tag=f"kv{dt}", name=f"kvt{dt}")
                        eng = (nc.scalar if not ringsplit or dt % 2 == 0
                               else nc.sync)
                        eng.dma_start(out=kvt_t, in_=kvT[dt * 128:(dt + 1) * 128, :])
                        kvts.append(kvt_t)
                    xts = None
                if dma_only and not bigdma:
                    for dt in range(NDT):
                        xt_t = xin.tile([128, SQ], BF16, tag="xt", name=f"xt{dt}")
                        nc.sync.dma_start(out=xt_t, in_=xT[dt * 128:(dt + 1) * 128, :])

                with tc.tile_pool(name="pqk", bufs=1, space="PSUM") as pqk:
                  if not dma_only:
                    # Q^T: dq-tile-outer accumulation into 8 resident psum banks
                    psq = [pqk.tile([128, QC], F32, tag=f"pq{i}", name=f"psq{i}")
                           for i in range(8)]
                    for dt in range(NDT):
                        if bigdma:
                            xt_t = xts[dt]
                        else:
                            xt_t = xin.tile([128, SQ], BF16, tag="xt", name=f"xt{dt}")
                            eng = (nc.sync if not ringsplit or dt % 2 == 0
                                   else nc.scalar)
                            eng.dma_start(out=xt_t, in_=xT[dt * 128:(dt + 1) * 128, :])
                        for i in range(8):
                            mh, qc = i // NQC, i % NQC
                            nc.tensor.matmul(psq[i],
                                             wq_sb[:, dt, mh * 128:(mh + 1) * 128],
                                             xt_t[:, qc * QC:(qc + 1) * QC],
                                             start=(dt == 0), stop=(dt == NDT - 1))
                    for i in range(8):
                        mh, qc = i // NQC, i % NQC
                        if actbias:
                            # ScalarE is idle in phase 1; Identity = in + bias
                            nc.scalar.activation(
                                out=QT[:, mh, qc * QC:(qc + 1) * QC], in_=psq[i],
                                func=mybir.ActivationFunctionType.Identity,
                                bias=bqk_sb[:, mh:mh + 1])
                        else:
                            nc.vector.tensor_scalar_add(
                                out=QT[:, mh, qc * QC:(qc + 1) * QC],
                                in0=psq[i], scalar1=bqk_sb[:, mh:mh + 1])
                    # K^T over resident kvT tiles; near-equal kv chunks <=512
                    ngrp = -(-nkt // 4)
                    chunks = []
                    t0 = 0
                    for gi in range(ngrp):
                        g = (nkt - t0 + (ngrp - gi - 1)) // (ngrp - gi)
                        chunks.append((t0 * 128, g * 128))
                        t0 += g
                    psk = []
                    for mh in range(2):
                        for ci, (c0, cl) in enumerate(chunks):
                            idx = mh * len(chunks) + ci
                            ps = pqk.tile([128, cl], F32, tag=f"pq{idx}",
                                          name=f"psk{idx}")
                            for dt in range(NDT):
                                nc.tensor.matmul(ps,
                                                 wk_sb[:, dt, mh * 128:(mh + 1) * 128],
                                                 kvts[dt][:, c0:c0 + cl],
                                                 start=(dt == 0), stop=(dt == NDT - 1))
                            psk.append((ps, mh, c0, cl))
                    for ps, mh, c0, cl in psk:
                        if actbias:
                            nc.scalar.activation(
                                out=KT[:, mh, c0:c0 + cl], in_=ps,
                                func=mybir.ActivationFunctionType.Identity,
                                bias=bqk_sb[:, 2 + mh:3 + mh])
                        else:
                            nc.vector.tensor_scalar_add(
                                out=KT[:, mh, c0:c0 + cl],
                                in0=ps, scalar1=bqk_sb[:, 2 + mh:3 + mh])

                # V-proj is deferred into phase 2 (hidden under the first exp
                # stream); make_vops builds its closures against a PSUM pool.
                def make_vops(pvpool):
                    ops = []
                    for t in range(nkt):
                        def op(t=t):
                            ps = pvpool.tile([128, CS], F32, tag="pv", name="psv")
                            for dt in range(NDT):
                                nc.tensor.matmul(ps,
                                                 kvts[dt][:, t * 128:(t + 1) * 128],
                                                 wv_sb[:, dt, :],
                                                 start=(dt == 0), stop=(dt == NDT - 1))
                            nc.vector.tensor_scalar(
                                out=V[:, t, :, 0:HD],
                                in0=ps.rearrange("p (h d) -> p h d", h=HPC),
                                scalar1=mcol_sb[:, t:t + 1], scalar2=None,
                                op0=MULT)
                        ops.append(op)
                    return ops

                if not dma_only and (not vdefer or "2" not in phases):
                    with tc.tile_pool(name="pv", bufs=4, space="PSUM") as pvp:
                        for op in make_vops(pvp):
                            op()

            # ---- Phase 2: attention, software-pipelined across head pairs ----
            with ExitStack() as _ph2:
              if "2" in phases and not dma_only:
                pp = _ph2.enter_context(tc.tile_pool(name="pp", bufs=1))
                outp = _ph2.enter_context(tc.tile_pool(name="outp", bufs=2))
                small = _ph2.enter_context(tc.tile_pool(name="small", bufs=2))
                psc = _ph2.enter_context(tc.tile_pool(name="psc", bufs=1, space="PSUM"))
                pools = {}

                copy_engines = {"vector": (nc.vector, nc.vector),
                                "gpsimd": (nc.gpsimd, nc.gpsimd),
                                "split": (nc.vector, nc.gpsimd)}[copy_eng]

                def norm_store(po_, h01, pr, qsl):
                    # 1/denominator (PSUM row 64) -> broadcast over 64 hd rows
                    rec = small.tile([HD + 1, QC], F32, tag="rec", name="rec")
                    nc.vector.reciprocal(out=rec[HD:HD + 1, :], in_=po_[HD:HD + 1, :])
                    rec0 = small.tile([1, QC], F32, tag="rec0", name="rec0")
                    nc.sync.dma_start(out=rec0[0:1, :], in_=rec[HD:HD + 1, :])
                    rb = small.tile([HD, QC], F32, tag="rb", name="rb")
                    nc.gpsimd.partition_broadcast(rb, rec0[0:1, :])
                    nc.vector.tensor_mul(out=OT[h01 * 64:(h01 + 1) * 64, pr, qsl],
                                         in0=po_[0:HD, :], in1=rb)

                def out_proj(st):
                    ot_sb = outp.tile([128, D], BF16 if obf16 else F32,
                                      tag="osb", name="ot_sb")
                    for nk in range(2):
                        ps = pools["pout"].tile([128, 512], F32, tag="po2",
                                                name="ps_out")
                        for ct in range(2):
                            nc.tensor.matmul(ps,
                                             OT[:, ct, st * 128:(st + 1) * 128],
                                             wo_sb[:, ct, nk * 512:(nk + 1) * 512],
                                             start=(ct == 0), stop=(ct == 1))
                        copy_engines[nk].tensor_copy(
                            out=ot_sb[:, nk * 512:(nk + 1) * 512], in_=ps)
                    nc.sync.dma_start(out=out_p[st * 128:(st + 1) * 128, :], in_=ot_sb)

                def pv_ops(qc, pr, P):
                    """Deferred-work closures for one head pair: 2x nkt PV
                    matmuls, 2 norm chains, and (after odd pairs) the 4
                    out-projections of the completed q-chunk."""
                    qsl = slice(qc * QC, (qc + 1) * QC)
                    ops = []
                    pos = [pools["pso"].tile([HD + 1, QC], F32, tag="po",
                                             name=f"po{qc}_{pr}_{h}")
                           for h in range(2)]
                    for h01 in range(2):
                        h = 2 * pr + h01
                        po_ = pos[h01]
                        for t in range(nkt):
                            ops.append((lambda po_=po_, t=t, h=h, h01=h01:
                                        nc.tensor.matmul(po_, V[:, t, h, :],
                                                         P[:, t, h01, :],
                                                         start=(t == 0),
                                                         stop=(t == nkt - 1))))
                        ops.append((lambda po_=po_, h01=h01:
                                    norm_store(po_, h01, pr, qsl)))
                    if pr == 1:
                        for st in range(qc * NQC, (qc + 1) * NQC):
                            ops.append(lambda st=st: out_proj(st))
                    return ops

                def st_exp_stream(qc, pr, P, carry_ops):
                    """Paired score matmuls + exp for head pair `pr` of chunk
                    `qc`, with the previous pair's deferred ops interleaved
                    ahead of each (possibly exp-stalled) score pair."""
                    qsl = slice(qc * QC, (qc + 1) * QC)
                    for t in range(nkt):
                        if interleave:
                            take = math.ceil(len(carry_ops) / (nkt - t))
                            for _ in range(take):
                                carry_ops.pop(0)()
                        ss = psc.tile([128, 2, QC], F32, tag="ss", bufs=2, name="ss")
                        nc.tensor.matmul(ss[:, 0, :],
                                         KT[0:64, pr, t * 128:(t + 1) * 128],
                                         QT[0:64, pr, qsl], start=True, stop=True)
                        nc.tensor.matmul(ss[:, 1, :],
                                         KT[64:128, pr, t * 128:(t + 1) * 128],
                                         QT[64:128, pr, qsl], start=True, stop=True)
                        nc.scalar.activation(out=P[:, t, :, :], in_=ss,
                                             func=EXP, scale=0.125)
                    for op in carry_ops:
                        op()

                def st_exp_stream_fused(qc, pr, P, carry_ops):
                    """Last combo: drain the previous pair's carry over the
                    first slots, then this pair's own PV rides behind exp so
                    only norm + out-proj remain after the final exp.  The PV
                    accumulators allocate only after the carry is fully
                    emitted (the pso pool cycles onto the previous pair's
                    buffers, whose readers must be in the program first)."""
                    qsl = slice(qc * QC, (qc + 1) * QC)
                    pos = None
                    fpv = 0
                    DR = min(4, nkt)

                    def catch_up(limit, final=False):
                        nonlocal fpv
                        while fpv < limit:
                            for h01 in range(2):
                                nc.tensor.matmul(pos[h01],
                                                 V[:, fpv, 2 * pr + h01, :],
                                                 P[:, fpv, h01, :],
                                                 start=(fpv == 0),
                                                 stop=(fpv == nkt - 1))
                            fpv += 1

                    for t in range(nkt):
                        if carry_ops and t < DR:
                            take = math.ceil(len(carry_ops) / (DR - t))
                            for _ in range(take):
                                carry_ops.pop(0)()
                        ss = psc.tile([128, 2, QC], F32, tag="ss", bufs=2, name="ss")
                        nc.tensor.matmul(ss[:, 0, :],
                                         KT[0:64, pr, t * 128:(t + 1) * 128],
                                         QT[0:64, pr, qsl], start=True, stop=True)
                        nc.tensor.matmul(ss[:, 1, :],
                                         KT[64:128, pr, t * 128:(t + 1) * 128],
                                         QT[64:128, pr, qsl], start=True, stop=True)
                        nc.scalar.activation(out=P[:, t, :, :], in_=ss,
                                             func=EXP, scale=0.125)
                        if pos is None and not carry_ops:
                            pos = [pools["pso"].tile([HD + 1, QC], F32, tag="po",
                                                     name=f"pof{h}")
                                   for h in range(2)]
                        if pos is not None:
                            catch_up(t)
                    for op in carry_ops:
                        op()
                    if pos is None:
                        pos = [pools["pso"].tile([HD + 1, QC], F32, tag="po",
                                                 name=f"pof{h}") for h in range(2)]
                    catch_up(nkt, final=True)
                    for h01 in range(2):
                        norm_store(pos[h01], h01, pr, qsl)
                    for st in range(qc * NQC, (qc + 1) * NQC):
                        out_proj(st)

                combos = [(qc, pr) for qc in range(NQC) for pr in range(2)]
                # The deferred V projection rides the first pair's exp stream;
                # its PSUM pool must close before pso/pout open (bank budget).
                vstack = ExitStack()
                if "1" in phases and vdefer:
                    pvp = vstack.enter_context(
                        tc.tile_pool(name="pv", bufs=2, space="PSUM"))
                    carry = make_vops(pvp)
                else:
                    carry = []
                for ci, (qc, pr) in enumerate(combos):
                    P = pp.tile([128, nkt, 2, QC], BF16, tag="P", bufs=p_bufs,
                                name=f"P{ci}")
                    if fuselast and ci == len(combos) - 1:
                        st_exp_stream_fused(qc, pr, P, carry)
                        carry = []
                        continue
                    st_exp_stream(qc, pr, P, carry)
                    if ci == 0:
                        vstack.close()
                        pools["pso"] = _ph2.enter_context(
                            tc.tile_pool(name="pso", bufs=2, space="PSUM"))
                        pools["pout"] = _ph2.enter_context(
                            tc.tile_pool(name="pout", bufs=2, space="PSUM"))
                    carry = pv_ops(qc, pr, P)
                for op in carry:
                    op()

            if loop_n > 1:
                loop_cm.__exit__(None, None, None)

    nc.compile()
    return nc


_NC = {}


def _get_nc(nkt):
    if nkt not in _NC:
        _NC[nkt] = _build(nkt)
    return _NC[nkt]


def _nkt_for(key_padding_mask):
    n = int((~np.asarray(key_padding_mask)).sum(axis=1).max())
    return min(max(-(-n // 128), 1), SKV // 128)


def _shard_inputs(query_input, key_value_input, key_padding_mask,
                  Wq, bq, Wk, bk, Wv, bv, Wo, bo):
    import ml_dtypes
    bf16 = ml_dtypes.bfloat16
    nkt = _nkt_for(key_padding_mask)
    skv = nkt * 128
    in_maps = []
    for c in range(NCORES):
        b, hg = c // (NCORES // B), c % (NCORES // B)
        cs = slice(hg * CS, (hg + 1) * CS)
        keep = ~key_padding_mask[b]
        n = int(keep.sum())
        kvc = np.zeros((skv, D), np.float32)
        kvc[:n] = key_value_input[b][keep]
        m01 = np.zeros((skv,), np.float32)
        m01[:n] = 1.0
        mcol = np.ascontiguousarray(m01.reshape(nkt, 128).T)          # [128, nkt]
        mones = np.ascontiguousarray(
            np.repeat(mcol[:, :, None], HPC, axis=2)).astype(bf16)    # [128, nkt, HPC]
        in_maps.append({
            "xT": np.ascontiguousarray(query_input[b].T).astype(bf16),
            "kvT": np.ascontiguousarray(kvc.T).astype(bf16),
            "wq": np.ascontiguousarray(
                Wq[:, cs].reshape(NDT, 128, CS).transpose(1, 0, 2)).astype(bf16),
            "wk": np.ascontiguousarray(
                Wk[:, cs].reshape(NDT, 128, CS).transpose(1, 0, 2)).astype(bf16),
            "wv": np.ascontiguousarray(
                Wv[:, cs].reshape(NDT, 128, CS).transpose(1, 0, 2)).astype(bf16),
            "wo": np.ascontiguousarray(
                Wo[cs, :].reshape(2, 128, D).transpose(1, 0, 2)).astype(bf16),
            "bqk": np.ascontiguousarray(
                np.stack([bq[cs][:128], bq[cs][128:], bk[cs][:128], bk[cs][128:]], axis=1)),
            "mcol": mcol,
            "mones": mones,
        })
    return in_maps


def kernel(query_input, key_value_input, key_padding_mask,
           Wq, bq, Wk, bk, Wv, bv, Wo, bo):
    query_input = np.asarray(query_input, np.float32)
    key_value_input = np.asarray(key_value_input, np.float32)
    key_padding_mask = np.asarray(key_padding_mask)
    Wq = np.asarray(Wq, np.float32); bq = np.asarray(bq, np.float32)
    Wk = np.asarray(Wk, np.float32); bk = np.asarray(bk, np.float32)
    Wv = np.asarray(Wv, np.float32); bv = np.asarray(bv, np.float32)
    Wo = np.asarray(Wo, np.float32); bo = np.asarray(bo, np.float32)

    nc = _get_nc(_nkt_for(key_padding_mask))
    in_maps = _shard_inputs(query_input, key_value_input, key_padding_mask,
                            Wq, bq, Wk, bk, Wv, bv, Wo, bo)
    res = run_bass_kernel_spmd(nc, in_maps, core_ids=list(range(NCORES)))

    # unshard: sum the 4 row-parallel partials per batch; V-bias contributes a
    # constant row (softmax rows sum to 1) folded in with bo here.
    const_row = (bv.astype(np.float64) @ Wo.astype(np.float64)) + bo.astype(np.float64)
    gpc = NCORES // B
    out = np.empty((B, SQ, D), np.float32)
    for b in range(B):
        acc = np.zeros((SQ, D), np.float64)
        for hg in range(gpc):
            acc += res.results[b * gpc + hg]["out_p"].astype(np.float64)
        out[b] = (acc + const_row[None, :]).astype(np.float32)
    return out
